# revision 8
# baseline (speedup 1.0000x reference)
"""Multi-head attention (LN -> QKV -> alibi attention -> out-proj) on 8 TRN2 cores.

Sharding: data-parallel over batch is replicated; heads are tensor-parallel:
core c computes heads {2c, 2c+1} for all batches, producing a partial
contribution to the output projection (its 128-row slice of D). Host sums the
8 partials and adds b_out.

Device pipeline per core (all 4 batches):
  A : LayerNorm stats + scaled rows xs = (x * rstd) with [murstd | 1 | 0pad]
      augmentation columns, written to a DRAM scratch (fp16).
  B1: DMA-transpose scratch -> xsT_aug [1152, 2048] (9 k-tiles, fp16).
  B2: QKV projections as matmuls with LN folded in via the augmented
      contraction (W rows: [g*W ; -colsum(gW) ; ln_b@W ; 0]).
  B3: per head: scoresT = kT^T q (j on partitions), alibi injected into PSUM
      via identity-matmul accumulate, exp on ScalarE -> p (fp16),
      PV matmul with ones-augmented V giving row sums for softmax.
  B4: partial out-projection with this core's 128 rows of w_out.
"""

import numpy as np
from contextlib import ExitStack

import concourse.bass as bass
import concourse.mybir as mybir
import concourse.tile as tile
from concourse import bacc
from concourse.bass_utils import run_bass_kernel_spmd
from concourse.masks import make_identity

B, N, D, H, DH = 4, 2048, 1024, 16, 64
N_CORES = 8
HPC = H // N_CORES          # heads per core = 2
SCALE = DH ** -0.5
EPS = 1e-5
EXP_SHIFT = 4.0             # exp(s - 4) keeps p comfortably inside fp16
KT = 9                      # contraction tiles: 8 x 128 (=D) + 1 aug tile
DAUG = KT * 128             # 1152
F16 = mybir.dt.float16
F32 = mybir.dt.float32

NT = N // 128               # 16 row tiles per batch
IH = 2                      # i-halves (1024 wide)
IW = N // IH                # 1024

PROFILE = False             # test.py sets True to collect HW exec time
LAST_RESULT = {}

_CACHE = {}


def build():
    nc = bacc.Bacc("TRN2", target_bir_lowering=False, debug=False,
                   num_devices=N_CORES)
    x_in = nc.dram_tensor("x", [B, N, D], F16, kind="ExternalInput").ap()
    alibiT = nc.dram_tensor("alibiT", [HPC, N, N], F16, kind="ExternalInput").ap()
    wbig = nc.dram_tensor("wbig", [DAUG, 6 * DH], F16, kind="ExternalInput").ap()
    wout = nc.dram_tensor("wout", [HPC * DH, D], F16, kind="ExternalInput").ap()
    outp = nc.dram_tensor("outp", [B, N, D], F16, kind="ExternalOutput").ap()

    with tile.TileContext(nc) as tc, ExitStack() as ctx:
        const = ctx.enter_context(tc.tile_pool(name="const", bufs=1))
        dramp = ctx.enter_context(tc.tile_pool(name="dram", bufs=2, space="DRAM"))
        apool = ctx.enter_context(tc.tile_pool(name="apool", bufs=3))
        spool = ctx.enter_context(tc.tile_pool(name="spool", bufs=4))
        xsp = ctx.enter_context(tc.tile_pool(name="xsp", bufs=1))
        qkp = ctx.enter_context(tc.tile_pool(name="qkp", bufs=2))
        vp = ctx.enter_context(tc.tile_pool(name="vp", bufs=2))
        alp = ctx.enter_context(tc.tile_pool(name="alp", bufs=4))
        pp = ctx.enter_context(tc.tile_pool(name="pp", bufs=3))
        atp = ctx.enter_context(tc.tile_pool(name="atp", bufs=2))
        ep = ctx.enter_context(tc.tile_pool(name="ep", bufs=2))
        outsb = ctx.enter_context(tc.tile_pool(name="outsb", bufs=4))

        # constants
        ident = const.tile([128, 128], F16, tag="ident")
        make_identity(nc, ident[:])
        eps_ap = const.tile([128, 1], F32, tag="eps")
        nc.gpsimd.memset(eps_ap[:], EPS)
        w_sb = []
        for kt in range(KT):
            t = const.tile([128, 6 * DH], F16, tag=f"w{kt}")
            nc.sync.dma_start(t[:], wbig[bass.ts(kt, 128), :])
            w_sb.append(t)
        wout_sb = const.tile([128, D], F16, tag="wout")
        nc.sync.dma_start(wout_sb[:], wout[:, :])

        for b in range(B):
            # ---------------- Phase A: LN + scaled rows -> DRAM scratch ----
            xs_dram = dramp.tile([N, DAUG], F16, tag="xs_dram")
            for nt in range(NT):
                xt = apool.tile([128, D], F16, tag="xt")
                nc.sync.dma_start(xt[:], x_in[b, bass.ts(nt, 128), :])
                stats = spool.tile([128, 2, 6], F32, tag="stats")
                xg = xt[:].rearrange("p (s f) -> p s f", f=512)
                nc.vector.bn_stats(out=stats[:, 0, :], in_=xg[:, 0, :])
                nc.vector.bn_stats(out=stats[:, 1, :], in_=xg[:, 1, :])
                mv = spool.tile([128, 2], F32, tag="mv")
                nc.vector.bn_aggr(out=mv[:], in_=stats[:])
                lnv = spool.tile([128, 1], F32, tag="lnv")
                nc.scalar.activation(lnv[:], mv[:, 1:2],
                                     mybir.ActivationFunctionType.Ln,
                                     bias=eps_ap[:])
                rstd = spool.tile([128, 1], F32, tag="rstd")
                nc.scalar.activation(rstd[:], lnv[:],
                                     mybir.ActivationFunctionType.Exp, scale=-0.5)
                xs = apool.tile([128, DAUG], F16, tag="xs")
                nc.gpsimd.tensor_scalar_mul(xs[:, 0:D], xt[:], rstd[:])
                # aug cols: murstd | 1 | zeros
                nc.vector.tensor_mul(xs[:, D:D + 1], mv[:, 0:1], rstd[:])
                nc.gpsimd.memset(xs[:, D + 1:D + 2], 1.0)
                nc.gpsimd.memset(xs[:, D + 2:DAUG], 0.0)
                nc.sync.dma_start(xs_dram[bass.ts(nt, 128), :], xs[:])

            # ---------------- Phase B1: transposed loads -------------------
            xsT = []
            for kt in range(KT):
                t = xsp.tile([128, N], F16, tag=f"xsT{kt}")
                nc.sync.dma_start(t[:], xs_dram[:, bass.ts(kt, 128)],
                                  transpose=True)
                xsT.append(t)

            # ---------------- Phase B2: QKV projections --------------------
            qk_sb = []   # per head: (qT, kT) [64, N] fp16
            for h in range(HPC):
                qT = qkp.tile([64, N], F16, tag=f"qT{h}")
                kT = qkp.tile([64, N], F16, tag=f"kT{h}")
                for c in range(4):
                    with tc.tile_pool(name=f"qkps{b}_{h}_{c}", bufs=1,
                                      space="PSUM") as ps:
                        acc = ps.tile([128, 512], F32)
                        for kt in range(KT):
                            nc.tensor.matmul(
                                acc[:],
                                w_sb[kt][:, bass.ds(h * 128, 128)],
                                xsT[kt][:, bass.ts(c, 512)],
                                start=(kt == 0), stop=(kt == KT - 1))
                        nc.vector.tensor_copy(qT[:, bass.ts(c, 512)], acc[0:64, :])
                        nc.vector.tensor_copy(kT[:, bass.ts(c, 512)], acc[64:128, :])
                qk_sb.append((qT, kT))

            v_sb = []    # 16 tiles [128, 130]: per head 64 v cols + ones col
            for nt in range(NT):
                va = vp.tile([128, 2 * (DH + 1)], F16, tag=f"v{nt}")
                with tc.tile_pool(name=f"vps{b}_{nt}", bufs=1,
                                  space="PSUM") as ps:
                    acc = ps.tile([128, 128], F32)
                    for kt in range(KT):
                        nc.tensor.matmul(acc[:], xsT[kt][:, bass.ts(nt, 128)],
                                         w_sb[kt][:, 256:384],
                                         start=(kt == 0), stop=(kt == KT - 1))
                    for h in range(HPC):
                        nc.vector.tensor_copy(
                            va[:, bass.ds(h * (DH + 1), DH)],
                            acc[:, bass.ds(h * DH, DH)])
                nc.gpsimd.memset(va[:, DH:DH + 1], 1.0)
                nc.gpsimd.memset(va[:, 2 * DH + 1:2 * DH + 2], 1.0)
                v_sb.append(va)

            # ---------------- Phase B3: attention --------------------------
            attnT = atp.tile([128, N], F16, tag="attnT")
            for h in range(HPC):
                qT, kT = qk_sb[h]
                for ih in range(IH):
                    with tc.tile_pool(name=f"pv{b}_{h}_{ih}", bufs=1,
                                      space="PSUM") as pvps, \
                         tc.tile_pool(name=f"sps{b}_{h}_{ih}", bufs=2,
                                      space="PSUM") as sps:
                        pv = pvps.tile([DH + 1, IW], F32)
                        for jt in range(NT):
                            al = alp.tile([128, IW], F16, tag="al")
                            nc.sync.dma_start(
                                al[:],
                                alibiT[h, bass.ts(jt, 128),
                                       bass.ds(ih * IW, IW)])
                            sp = sps.tile([128, IW], F32, tag="sp")
                            for c in range(2):
                                nc.tensor.matmul(
                                    sp[:, bass.ts(c, 512)],
                                    kT[:, bass.ts(jt, 128)],
                                    qT[:, bass.ds(ih * IW + c * 512, 512)],
                                    start=True, stop=False)
                                nc.tensor.matmul(
                                    sp[:, bass.ts(c, 512)],
                                    ident[:],
                                    al[:, bass.ts(c, 512)],
                                    start=False, stop=True)
                            p = pp.tile([128, IW], F16, tag="p")
                            nc.scalar.activation(
                                p[:], sp[:], mybir.ActivationFunctionType.Exp)
                            for c in range(2):
                                nc.tensor.matmul(
                                    pv[:, bass.ts(c, 512)],
                                    v_sb[jt][:, bass.ds(h * (DH + 1), DH + 1)],
                                    p[:, bass.ts(c, 512)],
                                    start=(jt == 0), stop=(jt == NT - 1))
                        # softmax normalize: attnT = pv[0:64] / pv[64]
                        srow = ep.tile([1, IW], F32, tag="srow")
                        nc.vector.tensor_copy(srow[:], pv[DH:DH + 1, :])
                        rec = ep.tile([1, IW], F16, tag="rec")
                        with nc.allow_low_precision(reason="softmax recip f16"):
                            nc.vector.reciprocal(rec[:], srow[:])
                        rb = ep.tile([64, IW], F16, tag="rb")
                        nc.gpsimd.partition_broadcast(rb[:], rec[:], channels=64)
                        nc.vector.tensor_mul(
                            attnT[bass.ds(h * DH, DH), bass.ds(ih * IW, IW)],
                            pv[0:DH, :], rb[:])

            # ---------------- Phase B4: out projection ---------------------
            for nt in range(NT):
                with tc.tile_pool(name=f"op{b}_{nt}", bufs=1,
                                  space="PSUM") as ops:
                    acc = ops.tile([128, D], F32)
                    for mc in range(2):
                        nc.tensor.matmul(acc[:, bass.ts(mc, 512)],
                                         attnT[:, bass.ts(nt, 128)],
                                         wout_sb[:, bass.ts(mc, 512)],
                                         start=True, stop=True)
                    ot = outsb.tile([128, D], F16, tag="ot")
                    nc.vector.tensor_copy(ot[:], acc[:])
                nc.sync.dma_start(outp[b, bass.ts(nt, 128), :], ot[:])

    nc.compile()
    return nc


def _get_nc():
    if "nc" not in _CACHE:
        _CACHE["nc"] = build()
    return _CACHE["nc"]


def kernel(x, alibi, w_qkv, w_out, b_out, ln_g, ln_b):
    x = np.asarray(x, dtype=np.float32)
    alibi = np.asarray(alibi, dtype=np.float32)
    w_qkv = np.asarray(w_qkv, dtype=np.float32)
    w_out = np.asarray(w_out, dtype=np.float32)
    b_out = np.asarray(b_out, dtype=np.float32)
    ln_g = np.asarray(ln_g, dtype=np.float32)
    ln_b = np.asarray(ln_b, dtype=np.float32)

    # fold LN gain + attention scale into the QKV weight; LN bias becomes an
    # extra row via the augmented contraction.
    W = w_qkv * ln_g[:, None]
    W[:, :D] *= SCALE
    c_row = ln_b @ w_qkv
    c_row[:D] *= SCALE
    colsum = W.sum(axis=0)

    x16 = x.astype(np.float16)
    in_maps = []
    for core in range(N_CORES):
        hs = [HPC * core + i for i in range(HPC)]
        cols = []
        for h in hs:
            cols.extend(range(h * DH, (h + 1) * DH))           # q
            cols.extend(range(D + h * DH, D + (h + 1) * DH))   # k
        vcols = []
        for h in hs:
            vcols.extend(range(2 * D + h * DH, 2 * D + (h + 1) * DH))
        cols = cols + vcols  # [q0|k0|q1|k1|v0|v1] -> 6*DH columns
        wbig = np.zeros((DAUG, 6 * DH), dtype=np.float32)
        wbig[:D, :] = W[:, cols]
        wbig[D, :] = -colsum[cols]
        wbig[D + 1, :] = c_row[cols]
        alT = np.ascontiguousarray(
            alibi[hs].transpose(0, 2, 1)) - np.float32(EXP_SHIFT)
        in_maps.append({
            "x": x16,
            "alibiT": alT.astype(np.float16),
            "wbig": wbig.astype(np.float16),
            "wout": w_out[hs[0] * DH: hs[0] * DH + HPC * DH, :]
                    .astype(np.float16),
        })

    nc = _get_nc()
    res = run_bass_kernel_spmd(nc, in_maps, list(range(N_CORES)),
                               trace=PROFILE)
    LAST_RESULT["exec_time_ns"] = res.exec_time_ns
    LAST_RESULT["mean_exec_time_ns"] = res.mean_exec_time_ns
    LAST_RESULT["instructions_and_trace"] = res.instructions_and_trace

    out = np.zeros((B, N, D), dtype=np.float32)
    for core in range(N_CORES):
        out += res.results[core]["outp"].astype(np.float32)
    out += b_out
    return out


# revision 13
# speedup vs baseline: 1.4717x; 1.4717x over previous
"""Multi-head attention (LN -> QKV -> alibi attention -> out-proj) on 8 TRN2 cores.

Sharding: data-parallel over batch is replicated; heads are tensor-parallel:
core c computes heads {2c, 2c+1} for all batches, producing a partial
contribution to the output projection (its 128-row slice of D). Host sums the
8 partials and adds b_out.

Device pipeline per core (all 4 batches):
  A : LayerNorm stats + scaled rows xs = (x * rstd) with [murstd | 1 | 0pad]
      augmentation columns, written to a DRAM scratch (fp16).
  B1: DMA-transpose scratch -> xsT_aug [1152, 2048] (9 k-tiles, fp16).
  B2: QKV projections as matmuls with LN folded in via the augmented
      contraction (W rows: [g*W ; -colsum(gW) ; ln_b@W ; 0]).
  B3: per head: scoresT = kT^T q (j on partitions), alibi injected into PSUM
      via identity-matmul accumulate, exp on ScalarE -> p (fp16),
      PV matmul with ones-augmented V giving row sums for softmax.
  B4: partial out-projection with this core's 128 rows of w_out.
"""

import numpy as np
from contextlib import ExitStack

import concourse.bass as bass
import concourse.mybir as mybir
import concourse.tile as tile
from concourse import bacc
from concourse.bass_utils import run_bass_kernel_spmd
from concourse.masks import make_identity
from concourse import bacc as _bacc_mod
from concourse import hw_specs as _hw_specs

_orig_gat = _hw_specs.get_activation_tables


def _gat_unified(arch):
    tabs = _orig_gat(arch)
    pref = "natural_log_exp_and_others"
    for name, funcs in tabs.items():
        if name != pref:
            funcs.discard(mybir.ActivationFunctionType.Exp)
            funcs.discard(mybir.ActivationFunctionType.Ln)
    return tabs


_bacc_mod.get_activation_tables = _gat_unified

B, N, D, H, DH = 4, 2048, 1024, 16, 64
N_CORES = 8
HPC = H // N_CORES          # heads per core = 2
SCALE = DH ** -0.5
EPS = 1e-5
EXP_SHIFT = 4.0             # exp(s - 4) keeps p comfortably inside fp16
KT = 9                      # contraction tiles: 8 x 128 (=D) + 1 aug tile
DAUG = KT * 128             # 1152
F16 = mybir.dt.float16
F32 = mybir.dt.float32

NT = N // 128               # 16 row tiles per batch
IH = 2                      # i-halves (1024 wide)
IW = N // IH                # 1024

PROFILE = False             # test.py sets True to collect HW exec time
LAST_RESULT = {}

_CACHE = {}


def build():
    nc = bacc.Bacc("TRN2", target_bir_lowering=False, debug=False,
                   num_devices=N_CORES)
    x_in = nc.dram_tensor("x", [B, N, D], F16, kind="ExternalInput").ap()
    alibiT = nc.dram_tensor("alibiT", [HPC, N, N], F16, kind="ExternalInput").ap()
    wbig = nc.dram_tensor("wbig", [DAUG, 6 * DH], F16, kind="ExternalInput").ap()
    wout = nc.dram_tensor("wout", [HPC * DH, D], F16, kind="ExternalInput").ap()
    outp = nc.dram_tensor("outp", [B, N, D], F16, kind="ExternalOutput").ap()

    with tile.TileContext(nc) as tc, ExitStack() as ctx:
        const = ctx.enter_context(tc.tile_pool(name="const", bufs=1))
        dramp = ctx.enter_context(tc.tile_pool(name="dram", bufs=2, space="DRAM"))
        apool = ctx.enter_context(tc.tile_pool(name="apool", bufs=3))
        spool = ctx.enter_context(tc.tile_pool(name="spool", bufs=4))
        xsp = ctx.enter_context(tc.tile_pool(name="xsp", bufs=1))
        qkp = ctx.enter_context(tc.tile_pool(name="qkp", bufs=2))
        vp = ctx.enter_context(tc.tile_pool(name="vp", bufs=2))
        alp = ctx.enter_context(tc.tile_pool(name="alp", bufs=4))
        pp = ctx.enter_context(tc.tile_pool(name="pp", bufs=3))
        atp = ctx.enter_context(tc.tile_pool(name="atp", bufs=2))
        ep = ctx.enter_context(tc.tile_pool(name="ep", bufs=2))
        outsb = ctx.enter_context(tc.tile_pool(name="outsb", bufs=4))

        # constants
        ident = const.tile([128, 128], F16, tag="ident")
        make_identity(nc, ident[:])
        eps_ap = const.tile([128, 1], F32, tag="eps")
        nc.gpsimd.memset(eps_ap[:], EPS)
        w_sb = []
        for kt in range(KT):
            t = const.tile([128, 6 * DH], F16, tag=f"w{kt}")
            nc.sync.dma_start(t[:], wbig[bass.ts(kt, 128), :])
            w_sb.append(t)
        wout_sb = const.tile([128, D], F16, tag="wout")
        nc.sync.dma_start(wout_sb[:], wout[:, :])

        for b in range(B):
            # ---------------- Phase A: LN + scaled rows -> DRAM scratch ----
            xs_dram = dramp.tile([N, DAUG], F16, tag="xs_dram")
            for nt in range(NT):
                xt = apool.tile([128, D], F16, tag="xt")
                nc.sync.dma_start(xt[:], x_in[b, bass.ts(nt, 128), :])
                stats = spool.tile([128, 2, 6], F32, tag="stats")
                xg = xt[:].rearrange("p (s f) -> p s f", f=512)
                nc.vector.bn_stats(out=stats[:, 0, :], in_=xg[:, 0, :])
                nc.vector.bn_stats(out=stats[:, 1, :], in_=xg[:, 1, :])
                mv = spool.tile([128, 2], F32, tag="mv")
                nc.vector.bn_aggr(out=mv[:], in_=stats[:])
                lnv = spool.tile([128, 1], F32, tag="lnv")
                nc.scalar.activation(lnv[:], mv[:, 1:2],
                                     mybir.ActivationFunctionType.Ln,
                                     bias=eps_ap[:])
                rstd = spool.tile([128, 1], F32, tag="rstd")
                nc.scalar.activation(rstd[:], lnv[:],
                                     mybir.ActivationFunctionType.Exp, scale=-0.5)
                xs = apool.tile([128, DAUG], F16, tag="xs")
                nc.vector.tensor_scalar_mul(xs[:, 0:D], xt[:], rstd[:])
                # aug cols: murstd | 1 | zeros
                nc.scalar.mul(xs[:, D:D + 1], mv[:, 0:1], rstd[:])
                nc.gpsimd.memset(xs[:, D + 1:D + 2], 1.0)
                nc.gpsimd.memset(xs[:, D + 2:DAUG], 0.0)
                nc.sync.dma_start(xs_dram[bass.ts(nt, 128), :], xs[:])

            # ---------------- Phase B1: transposed loads -------------------
            xsT = []
            for kt in range(KT):
                t = xsp.tile([128, N], F16, tag=f"xsT{kt}")
                nc.sync.dma_start(t[:], xs_dram[:, bass.ts(kt, 128)],
                                  transpose=True)
                xsT.append(t)

            # ---------------- Phase B2: QKV projections --------------------
            qk_sb = []   # per head: (qT, kT) [64, N] fp16
            for h in range(HPC):
                qT = qkp.tile([64, N], F16, tag=f"qT{h}")
                kT = qkp.tile([64, N], F16, tag=f"kT{h}")
                for c in range(4):
                    with tc.tile_pool(name=f"qkps{b}_{h}_{c}", bufs=1,
                                      space="PSUM") as ps:
                        acc = ps.tile([128, 512], F32)
                        for kt in range(KT):
                            nc.tensor.matmul(
                                acc[:],
                                w_sb[kt][:, bass.ds(h * 128, 128)],
                                xsT[kt][:, bass.ts(c, 512)],
                                start=(kt == 0), stop=(kt == KT - 1))
                        nc.vector.tensor_copy(qT[:, bass.ts(c, 512)], acc[0:64, :])
                        nc.vector.tensor_copy(kT[:, bass.ts(c, 512)], acc[64:128, :])
                qk_sb.append((qT, kT))

            v_sb = []    # 16 tiles [128, 130]: per head 64 v cols + ones col
            for nt in range(NT):
                va = vp.tile([128, 2 * (DH + 1)], F16, tag=f"v{nt}")
                with tc.tile_pool(name=f"vps{b}_{nt}", bufs=1,
                                  space="PSUM") as ps:
                    acc = ps.tile([128, 128], F32)
                    for kt in range(KT):
                        nc.tensor.matmul(acc[:], xsT[kt][:, bass.ts(nt, 128)],
                                         w_sb[kt][:, 256:384],
                                         start=(kt == 0), stop=(kt == KT - 1))
                    for h in range(HPC):
                        nc.vector.tensor_copy(
                            va[:, bass.ds(h * (DH + 1), DH)],
                            acc[:, bass.ds(h * DH, DH)])
                nc.gpsimd.memset(va[:, DH:DH + 1], 1.0)
                nc.gpsimd.memset(va[:, 2 * DH + 1:2 * DH + 2], 1.0)
                v_sb.append(va)

            # ---------------- Phase B3: attention --------------------------
            attnT = atp.tile([128, N], F16, tag="attnT")
            for h in range(HPC):
                qT, kT = qk_sb[h]
                for ih in range(IH):
                    with tc.tile_pool(name=f"pv{b}_{h}_{ih}", bufs=1,
                                      space="PSUM") as pvps, \
                         tc.tile_pool(name=f"sps{b}_{h}_{ih}", bufs=2,
                                      space="PSUM") as sps:
                        pv = pvps.tile([DH + 1, IW], F32)
                        for jt in range(NT):
                            al = alp.tile([128, IW], F16, tag="al")
                            nc.sync.dma_start(
                                al[:],
                                alibiT[h, bass.ts(jt, 128),
                                       bass.ds(ih * IW, IW)])
                            sp = sps.tile([128, IW], F32, tag="sp")
                            for c in range(2):
                                nc.tensor.matmul(
                                    sp[:, bass.ts(c, 512)],
                                    kT[:, bass.ts(jt, 128)],
                                    qT[:, bass.ds(ih * IW + c * 512, 512)],
                                    start=True, stop=False)
                                nc.tensor.matmul(
                                    sp[:, bass.ts(c, 512)],
                                    ident[:],
                                    al[:, bass.ts(c, 512)],
                                    start=False, stop=True)
                            p = pp.tile([128, IW], F16, tag="p")
                            nc.scalar.activation(
                                p[:], sp[:], mybir.ActivationFunctionType.Exp)
                            for c in range(2):
                                nc.tensor.matmul(
                                    pv[:, bass.ts(c, 512)],
                                    v_sb[jt][:, bass.ds(h * (DH + 1), DH + 1)],
                                    p[:, bass.ts(c, 512)],
                                    start=(jt == 0), stop=(jt == NT - 1))
                        # softmax normalize: attnT = pv[0:64] / pv[64]
                        srow = ep.tile([1, IW], F32, tag="srow")
                        nc.vector.tensor_copy(srow[:], pv[DH:DH + 1, :])
                        sdram = dramp.tile([1, IW], F32, tag="sdram")
                        nc.sync.dma_start(sdram[:], srow[:])
                        rr = ep.tile([128, IW // 128], F32, tag="rr")
                        nc.sync.dma_start(
                            rr[:], sdram[0, :].rearrange("(p f) -> p f", p=128))
                        rr16 = ep.tile([128, IW // 128], F16, tag="rr16")
                        with nc.allow_low_precision(reason="softmax recip f16"):
                            nc.vector.reciprocal(rr16[:], rr[:])
                        rdram = dramp.tile([1, IW], F16, tag="rdram")
                        nc.sync.dma_start(
                            rdram[0, :].rearrange("(p f) -> p f", p=128), rr16[:])
                        rec = ep.tile([1, IW], F16, tag="rec")
                        nc.sync.dma_start(rec[:], rdram[:])
                        rb = ep.tile([64, IW], F16, tag="rb")
                        nc.gpsimd.partition_broadcast(rb[:], rec[:], channels=64)
                        nc.vector.tensor_mul(
                            attnT[bass.ds(h * DH, DH), bass.ds(ih * IW, IW)],
                            pv[0:DH, :], rb[:])

            # ---------------- Phase B4: out projection ---------------------
            for nt in range(NT):
                with tc.tile_pool(name=f"op{b}_{nt}", bufs=1,
                                  space="PSUM") as ops:
                    acc = ops.tile([128, D], F32)
                    for mc in range(2):
                        nc.tensor.matmul(acc[:, bass.ts(mc, 512)],
                                         attnT[:, bass.ts(nt, 128)],
                                         wout_sb[:, bass.ts(mc, 512)],
                                         start=True, stop=True)
                    ot = outsb.tile([128, D], F16, tag="ot")
                    nc.vector.tensor_copy(ot[:], acc[:])
                nc.sync.dma_start(outp[b, bass.ts(nt, 128), :], ot[:])

    nc.compile()
    return nc


def _get_nc():
    if "nc" not in _CACHE:
        _CACHE["nc"] = build()
    return _CACHE["nc"]


def kernel(x, alibi, w_qkv, w_out, b_out, ln_g, ln_b):
    x = np.asarray(x, dtype=np.float32)
    alibi = np.asarray(alibi, dtype=np.float32)
    w_qkv = np.asarray(w_qkv, dtype=np.float32)
    w_out = np.asarray(w_out, dtype=np.float32)
    b_out = np.asarray(b_out, dtype=np.float32)
    ln_g = np.asarray(ln_g, dtype=np.float32)
    ln_b = np.asarray(ln_b, dtype=np.float32)

    # fold LN gain + attention scale into the QKV weight; LN bias becomes an
    # extra row via the augmented contraction.
    W = w_qkv * ln_g[:, None]
    W[:, :D] *= SCALE
    c_row = ln_b @ w_qkv
    c_row[:D] *= SCALE
    colsum = W.sum(axis=0)

    x16 = x.astype(np.float16)
    in_maps = []
    for core in range(N_CORES):
        hs = [HPC * core + i for i in range(HPC)]
        cols = []
        for h in hs:
            cols.extend(range(h * DH, (h + 1) * DH))           # q
            cols.extend(range(D + h * DH, D + (h + 1) * DH))   # k
        vcols = []
        for h in hs:
            vcols.extend(range(2 * D + h * DH, 2 * D + (h + 1) * DH))
        cols = cols + vcols  # [q0|k0|q1|k1|v0|v1] -> 6*DH columns
        wbig = np.zeros((DAUG, 6 * DH), dtype=np.float32)
        wbig[:D, :] = W[:, cols]
        wbig[D, :] = -colsum[cols]
        wbig[D + 1, :] = c_row[cols]
        alT = np.ascontiguousarray(
            alibi[hs].transpose(0, 2, 1)) - np.float32(EXP_SHIFT)
        in_maps.append({
            "x": x16,
            "alibiT": alT.astype(np.float16),
            "wbig": wbig.astype(np.float16),
            "wout": w_out[hs[0] * DH: hs[0] * DH + HPC * DH, :]
                    .astype(np.float16),
        })

    nc = _get_nc()
    res = run_bass_kernel_spmd(nc, in_maps, list(range(N_CORES)),
                               trace=PROFILE)
    LAST_RESULT["exec_time_ns"] = res.exec_time_ns
    LAST_RESULT["mean_exec_time_ns"] = res.mean_exec_time_ns
    LAST_RESULT["instructions_and_trace"] = res.instructions_and_trace

    out = np.zeros((B, N, D), dtype=np.float32)
    for core in range(N_CORES):
        out += res.results[core]["outp"].astype(np.float32)
    out += b_out
    return out


# revision 15
# speedup vs baseline: 1.6531x; 1.1233x over previous
"""Multi-head attention (LN -> QKV -> alibi attention -> out-proj) on 8 TRN2 cores.

Sharding: data-parallel over batch is replicated; heads are tensor-parallel:
core c computes heads {2c, 2c+1} for all batches, producing a partial
contribution to the output projection (its 128-row slice of D). Host sums the
8 partials and adds b_out.

Device pipeline per core (all 4 batches):
  A : LayerNorm stats + scaled rows xs = (x * rstd) with [murstd | 1 | 0pad]
      augmentation columns, written to a DRAM scratch (fp16).
  B1: DMA-transpose scratch -> xsT_aug [1152, 2048] (9 k-tiles, fp16).
  B2: QKV projections as matmuls with LN folded in via the augmented
      contraction (W rows: [g*W ; -colsum(gW) ; ln_b@W ; 0]).
  B3: per head: scoresT = kT^T q (j on partitions), alibi injected into PSUM
      via identity-matmul accumulate, exp on ScalarE -> p (fp16),
      PV matmul with ones-augmented V giving row sums for softmax.
  B4: partial out-projection with this core's 128 rows of w_out.
"""

import numpy as np
from contextlib import ExitStack

import concourse.bass as bass
import concourse.mybir as mybir
import concourse.tile as tile
from concourse import bacc
from concourse.bass_utils import run_bass_kernel_spmd
from concourse.masks import make_identity
from concourse import bacc as _bacc_mod
from concourse import hw_specs as _hw_specs

_orig_gat = _hw_specs.get_activation_tables


def _gat_unified(arch):
    tabs = _orig_gat(arch)
    pref = "natural_log_exp_and_others"
    for name, funcs in tabs.items():
        if name != pref:
            funcs.discard(mybir.ActivationFunctionType.Exp)
            funcs.discard(mybir.ActivationFunctionType.Ln)
    return tabs


_bacc_mod.get_activation_tables = _gat_unified

B, N, D, H, DH = 4, 2048, 1024, 16, 64
N_CORES = 8
HPC = H // N_CORES          # heads per core = 2
SCALE = DH ** -0.5
EPS = 1e-5
EXP_SHIFT = 4.0             # exp(s - 4) keeps p comfortably inside fp16
KT = 9                      # contraction tiles: 8 x 128 (=D) + 1 aug tile
DAUG = KT * 128             # 1152
F16 = mybir.dt.float16
F32 = mybir.dt.float32

NT = N // 128               # 16 row tiles per batch
IH = 2                      # i-halves (1024 wide)
IW = N // IH                # 1024

PROFILE = False             # test.py sets True to collect HW exec time
LAST_RESULT = {}

_CACHE = {}


def build():
    nc = bacc.Bacc("TRN2", target_bir_lowering=False, debug=False,
                   num_devices=N_CORES)
    x_in = nc.dram_tensor("x", [B, N, D], F16, kind="ExternalInput").ap()
    alibiT = nc.dram_tensor("alibiT", [HPC, N, N], F16, kind="ExternalInput").ap()
    wbig = nc.dram_tensor("wbig", [DAUG, 6 * DH], F16, kind="ExternalInput").ap()
    wout = nc.dram_tensor("wout", [HPC * DH, D], F16, kind="ExternalInput").ap()
    outp = nc.dram_tensor("outp", [B, N, D], F16, kind="ExternalOutput").ap()

    with tile.TileContext(nc) as tc, ExitStack() as ctx:
        const = ctx.enter_context(tc.tile_pool(name="const", bufs=1))
        dramp = ctx.enter_context(tc.tile_pool(name="dram", bufs=2, space="DRAM"))
        apool = ctx.enter_context(tc.tile_pool(name="apool", bufs=3))
        spool = ctx.enter_context(tc.tile_pool(name="spool", bufs=4))
        xsp = ctx.enter_context(tc.tile_pool(name="xsp", bufs=1))
        qkp = ctx.enter_context(tc.tile_pool(name="qkp", bufs=2))
        vp = ctx.enter_context(tc.tile_pool(name="vp", bufs=2))
        alp = ctx.enter_context(tc.tile_pool(name="alp", bufs=4))
        pp = ctx.enter_context(tc.tile_pool(name="pp", bufs=3))
        atp = ctx.enter_context(tc.tile_pool(name="atp", bufs=2))
        ep = ctx.enter_context(tc.tile_pool(name="ep", bufs=2))
        outsb = ctx.enter_context(tc.tile_pool(name="outsb", bufs=4))

        # constants
        ident = const.tile([128, 128], F16, tag="ident")
        make_identity(nc, ident[:])
        eps_ap = const.tile([128, 1], F32, tag="eps")
        nc.gpsimd.memset(eps_ap[:], EPS)
        w_sb = []
        for kt in range(KT):
            t = const.tile([128, 6 * DH], F16, tag=f"w{kt}")
            nc.sync.dma_start(t[:], wbig[bass.ts(kt, 128), :])
            w_sb.append(t)
        wout_sb = const.tile([128, D], F16, tag="wout")
        nc.sync.dma_start(wout_sb[:], wout[:, :])

        for b in range(B):
            # ---------------- Phase A: LN + scaled rows -> DRAM scratch ----
            xs_dram = dramp.tile([N, DAUG], F16, tag="xs_dram")
            for nt in range(NT):
                xt = apool.tile([128, D], F16, tag="xt")
                nc.sync.dma_start(xt[:], x_in[b, bass.ts(nt, 128), :])
                stats = spool.tile([128, 2, 6], F32, tag="stats")
                xg = xt[:].rearrange("p (s f) -> p s f", f=512)
                nc.vector.bn_stats(out=stats[:, 0, :], in_=xg[:, 0, :])
                nc.vector.bn_stats(out=stats[:, 1, :], in_=xg[:, 1, :])
                mv = spool.tile([128, 2], F32, tag="mv")
                nc.vector.bn_aggr(out=mv[:], in_=stats[:])
                lnv = spool.tile([128, 1], F32, tag="lnv")
                nc.scalar.activation(lnv[:], mv[:, 1:2],
                                     mybir.ActivationFunctionType.Ln,
                                     bias=eps_ap[:])
                rstd = spool.tile([128, 1], F32, tag="rstd")
                nc.scalar.activation(rstd[:], lnv[:],
                                     mybir.ActivationFunctionType.Exp, scale=-0.5)
                xs = apool.tile([128, DAUG], F16, tag="xs")
                nc.vector.tensor_scalar_mul(xs[:, 0:D], xt[:], rstd[:])
                # aug cols: murstd | 1 | zeros
                nc.scalar.mul(xs[:, D:D + 1], mv[:, 0:1], rstd[:])
                nc.gpsimd.memset(xs[:, D + 1:D + 2], 1.0)
                nc.gpsimd.memset(xs[:, D + 2:DAUG], 0.0)
                nc.sync.dma_start(xs_dram[bass.ts(nt, 128), :], xs[:])

            # ---------------- Phase B1: transposed loads -------------------
            xsT = []
            for kt in range(KT):
                t = xsp.tile([128, N], F16, tag=f"xsT{kt}")
                nc.sync.dma_start(t[:], xs_dram[:, bass.ts(kt, 128)],
                                  transpose=True)
                xsT.append(t)

            # ---------------- Phase B2: QKV projections --------------------
            qk_sb = []   # per head: (qT, kT) [64, N] fp16
            for h in range(HPC):
                qT = qkp.tile([64, N], F16, tag=f"qT{h}")
                kT = qkp.tile([64, N], F16, tag=f"kT{h}")
                for c in range(4):
                    with tc.tile_pool(name=f"qkps{b}_{h}_{c}", bufs=1,
                                      space="PSUM") as ps:
                        acc = ps.tile([128, 512], F32)
                        for kt in range(KT):
                            nc.tensor.matmul(
                                acc[:],
                                w_sb[kt][:, bass.ds(h * 128, 128)],
                                xsT[kt][:, bass.ts(c, 512)],
                                start=(kt == 0), stop=(kt == KT - 1))
                        nc.vector.tensor_copy(qT[:, bass.ts(c, 512)], acc[0:64, :])
                        nc.vector.tensor_copy(kT[:, bass.ts(c, 512)], acc[64:128, :])
                qk_sb.append((qT, kT))

            v_sb = []    # 16 tiles [128, 130]: per head 64 v cols + ones col
            for nt in range(NT):
                va = vp.tile([128, 2 * (DH + 1)], F16, tag=f"v{nt}")
                with tc.tile_pool(name=f"vps{b}_{nt}", bufs=1,
                                  space="PSUM") as ps:
                    acc = ps.tile([128, 128], F32)
                    for kt in range(KT):
                        nc.tensor.matmul(acc[:], xsT[kt][:, bass.ts(nt, 128)],
                                         w_sb[kt][:, 256:384],
                                         start=(kt == 0), stop=(kt == KT - 1))
                    for h in range(HPC):
                        nc.vector.tensor_copy(
                            va[:, bass.ds(h * (DH + 1), DH)],
                            acc[:, bass.ds(h * DH, DH)])
                nc.gpsimd.memset(va[:, DH:DH + 1], 1.0)
                nc.gpsimd.memset(va[:, 2 * DH + 1:2 * DH + 2], 1.0)
                v_sb.append(va)

            # ---------------- Phase B3: attention --------------------------
            attnT = atp.tile([128, N], F16, tag="attnT")
            for ih in range(IH):
                with tc.tile_pool(name=f"pv{b}_{ih}", bufs=2,
                                  space="PSUM") as pvps, \
                     tc.tile_pool(name=f"sps{b}_{ih}", bufs=2,
                                  space="PSUM") as sps:
                    pv = [pvps.tile([DH + 1, IW], F32, name=f"pv{h}",
                                    tag="pv") for h in range(HPC)]
                    for jt in range(NT):
                        als = []
                        for h in range(HPC):
                            al = alp.tile([128, IW], F16, tag="al")
                            nc.sync.dma_start(
                                al[:],
                                alibiT[h, bass.ts(jt, 128),
                                       bass.ds(ih * IW, IW)])
                            als.append(al)
                        for h in range(HPC):
                            qT, kT = qk_sb[h]
                            sp = sps.tile([128, IW], F32, tag="sp")
                            for c in range(2):
                                nc.tensor.matmul(
                                    sp[:, bass.ts(c, 512)],
                                    kT[:, bass.ts(jt, 128)],
                                    qT[:, bass.ds(ih * IW + c * 512, 512)],
                                    start=True, stop=False)
                                nc.tensor.matmul(
                                    sp[:, bass.ts(c, 512)],
                                    ident[:],
                                    als[h][:, bass.ts(c, 512)],
                                    start=False, stop=True)
                            p = pp.tile([128, IW], F16, tag="p")
                            nc.scalar.activation(
                                p[:], sp[:], mybir.ActivationFunctionType.Exp)
                            for c in range(2):
                                nc.tensor.matmul(
                                    pv[h][:, bass.ts(c, 512)],
                                    v_sb[jt][:, bass.ds(h * (DH + 1), DH + 1)],
                                    p[:, bass.ts(c, 512)],
                                    start=(jt == 0), stop=(jt == NT - 1))
                    # fast PSUM drain: copy out, normalize later from SBUF
                    for h in range(HPC):
                        un = ep.tile([64, IW], F32, tag="un")
                        nc.vector.tensor_copy(un[:], pv[h][0:DH, :])
                        srow = ep.tile([1, IW], F32, tag="srow")
                        nc.vector.tensor_copy(srow[:], pv[h][DH:DH + 1, :])
                        sdram = dramp.tile([1, IW], F32, tag="sdram")
                        nc.sync.dma_start(sdram[:], srow[:])
                        rr = ep.tile([128, IW // 128], F32, tag="rr")
                        nc.sync.dma_start(
                            rr[:], sdram[0, :].rearrange("(p f) -> p f", p=128))
                        rr16 = ep.tile([128, IW // 128], F16, tag="rr16")
                        with nc.allow_low_precision(reason="softmax recip f16"):
                            nc.vector.reciprocal(rr16[:], rr[:])
                        rdram = dramp.tile([1, IW], F16, tag="rdram")
                        nc.sync.dma_start(
                            rdram[0, :].rearrange("(p f) -> p f", p=128),
                            rr16[:])
                        rec = ep.tile([1, IW], F16, tag="rec")
                        nc.sync.dma_start(rec[:], rdram[:])
                        rb = ep.tile([64, IW], F16, tag="rb")
                        nc.gpsimd.partition_broadcast(rb[:], rec[:],
                                                      channels=64)
                        nc.vector.tensor_mul(
                            attnT[bass.ds(h * DH, DH), bass.ds(ih * IW, IW)],
                            un[:], rb[:])

            # ---------------- Phase B4: out projection ---------------------
            for nt in range(NT):
                with tc.tile_pool(name=f"op{b}_{nt}", bufs=1,
                                  space="PSUM") as ops:
                    acc = ops.tile([128, D], F32)
                    for mc in range(2):
                        nc.tensor.matmul(acc[:, bass.ts(mc, 512)],
                                         attnT[:, bass.ts(nt, 128)],
                                         wout_sb[:, bass.ts(mc, 512)],
                                         start=True, stop=True)
                    ot = outsb.tile([128, D], F16, tag="ot")
                    if nt % 2 == 0:
                        nc.vector.tensor_copy(ot[:], acc[:])
                    else:
                        nc.scalar.copy(ot[:], acc[:])
                nc.sync.dma_start(outp[b, bass.ts(nt, 128), :], ot[:])

    nc.compile()
    return nc


def _get_nc():
    if "nc" not in _CACHE:
        _CACHE["nc"] = build()
    return _CACHE["nc"]


def kernel(x, alibi, w_qkv, w_out, b_out, ln_g, ln_b):
    x = np.asarray(x, dtype=np.float32)
    alibi = np.asarray(alibi, dtype=np.float32)
    w_qkv = np.asarray(w_qkv, dtype=np.float32)
    w_out = np.asarray(w_out, dtype=np.float32)
    b_out = np.asarray(b_out, dtype=np.float32)
    ln_g = np.asarray(ln_g, dtype=np.float32)
    ln_b = np.asarray(ln_b, dtype=np.float32)

    # fold LN gain + attention scale into the QKV weight; LN bias becomes an
    # extra row via the augmented contraction.
    W = w_qkv * ln_g[:, None]
    W[:, :D] *= SCALE
    c_row = ln_b @ w_qkv
    c_row[:D] *= SCALE
    colsum = W.sum(axis=0)

    x16 = x.astype(np.float16)
    in_maps = []
    for core in range(N_CORES):
        hs = [HPC * core + i for i in range(HPC)]
        cols = []
        for h in hs:
            cols.extend(range(h * DH, (h + 1) * DH))           # q
            cols.extend(range(D + h * DH, D + (h + 1) * DH))   # k
        vcols = []
        for h in hs:
            vcols.extend(range(2 * D + h * DH, 2 * D + (h + 1) * DH))
        cols = cols + vcols  # [q0|k0|q1|k1|v0|v1] -> 6*DH columns
        wbig = np.zeros((DAUG, 6 * DH), dtype=np.float32)
        wbig[:D, :] = W[:, cols]
        wbig[D, :] = -colsum[cols]
        wbig[D + 1, :] = c_row[cols]
        alT = np.ascontiguousarray(
            alibi[hs].transpose(0, 2, 1)) - np.float32(EXP_SHIFT)
        in_maps.append({
            "x": x16,
            "alibiT": alT.astype(np.float16),
            "wbig": wbig.astype(np.float16),
            "wout": w_out[hs[0] * DH: hs[0] * DH + HPC * DH, :]
                    .astype(np.float16),
        })

    nc = _get_nc()
    res = run_bass_kernel_spmd(nc, in_maps, list(range(N_CORES)),
                               trace=PROFILE)
    LAST_RESULT["exec_time_ns"] = res.exec_time_ns
    LAST_RESULT["mean_exec_time_ns"] = res.mean_exec_time_ns
    LAST_RESULT["instructions_and_trace"] = res.instructions_and_trace

    out = np.zeros((B, N, D), dtype=np.float32)
    for core in range(N_CORES):
        out += res.results[core]["outp"].astype(np.float32)
    out += b_out
    return out


# revision 17
# speedup vs baseline: 2.1128x; 1.2781x over previous
"""Multi-head attention (LN -> QKV -> alibi attention -> out-proj) on 8 TRN2 cores.

Sharding: data-parallel over batch is replicated; heads are tensor-parallel:
core c computes heads {2c, 2c+1} for all batches, producing a partial
contribution to the output projection (its 128-row slice of D). Host sums the
8 partials and adds b_out.

Device pipeline per core (all 4 batches):
  A : LayerNorm stats + scaled rows xs = (x * rstd) with [murstd | 1 | 0pad]
      augmentation columns, written to a DRAM scratch (fp16).
  B1: DMA-transpose scratch -> xsT_aug [1152, 2048] (9 k-tiles, fp16).
  B2: QKV projections as matmuls with LN folded in via the augmented
      contraction (W rows: [g*W ; -colsum(gW) ; ln_b@W ; 0]).
  B3: per head: scoresT = kT^T q (j on partitions), alibi injected into PSUM
      via identity-matmul accumulate, exp on ScalarE -> p (fp16),
      PV matmul with ones-augmented V giving row sums for softmax.
  B4: partial out-projection with this core's 128 rows of w_out.
"""

import numpy as np
from contextlib import ExitStack

import concourse.bass as bass
import concourse.mybir as mybir
import concourse.tile as tile
from concourse import bacc
from concourse.bass_utils import run_bass_kernel_spmd
from concourse.masks import make_identity
from concourse import bacc as _bacc_mod
from concourse import hw_specs as _hw_specs

_orig_gat = _hw_specs.get_activation_tables


def _gat_unified(arch):
    tabs = _orig_gat(arch)
    pref = "natural_log_exp_and_others"
    for name, funcs in tabs.items():
        if name != pref:
            funcs.discard(mybir.ActivationFunctionType.Exp)
            funcs.discard(mybir.ActivationFunctionType.Ln)
    return tabs


_bacc_mod.get_activation_tables = _gat_unified

B, N, D, H, DH = 4, 2048, 1024, 16, 64
N_CORES = 8
HPC = H // N_CORES          # heads per core = 2
SCALE = DH ** -0.5
EPS = 1e-5
EXP_SHIFT = 4.0             # exp(s - 4) keeps p comfortably inside fp16
KT = 9                      # contraction tiles: 8 x 128 (=D) + 1 aug tile
DAUG = KT * 128             # 1152
F16 = mybir.dt.float16
F32 = mybir.dt.float32

NT = N // 128               # 16 row tiles per batch
IH = 2                      # i-halves (1024 wide)
IW = N // IH                # 1024

PROFILE = False             # test.py sets True to collect HW exec time
LAST_RESULT = {}

_CACHE = {}


def build():
    nc = bacc.Bacc("TRN2", target_bir_lowering=False, debug=False,
                   num_devices=N_CORES)
    x_in = nc.dram_tensor("x", [B, N, D], F16, kind="ExternalInput").ap()
    alibiT = nc.dram_tensor("alibiT", [HPC, N, N], F16, kind="ExternalInput").ap()
    wbig = nc.dram_tensor("wbig", [DAUG, 6 * DH], F16, kind="ExternalInput").ap()
    wout = nc.dram_tensor("wout", [HPC * DH, D], F16, kind="ExternalInput").ap()
    outp = nc.dram_tensor("outp", [B, N, D], F16, kind="ExternalOutput").ap()

    with tile.TileContext(nc) as tc, ExitStack() as ctx:
        const = ctx.enter_context(tc.tile_pool(name="const", bufs=1))
        dramp = ctx.enter_context(tc.tile_pool(name="dram", bufs=2, space="DRAM"))
        apool = ctx.enter_context(tc.tile_pool(name="apool", bufs=3))
        spool = ctx.enter_context(tc.tile_pool(name="spool", bufs=4))
        xsp = ctx.enter_context(tc.tile_pool(name="xsp", bufs=1))
        qkp = ctx.enter_context(tc.tile_pool(name="qkp", bufs=2))
        vp = ctx.enter_context(tc.tile_pool(name="vp", bufs=2))
        alp = ctx.enter_context(tc.tile_pool(name="alp", bufs=4))
        pp = ctx.enter_context(tc.tile_pool(name="pp", bufs=3))
        atp = ctx.enter_context(tc.tile_pool(name="atp", bufs=2))
        ep = ctx.enter_context(tc.tile_pool(name="ep", bufs=2))
        outsb = ctx.enter_context(tc.tile_pool(name="outsb", bufs=4))

        # constants
        ident = const.tile([128, 128], F16, tag="ident")
        make_identity(nc, ident[:])
        eps_ap = const.tile([128, 1], F32, tag="eps")
        nc.gpsimd.memset(eps_ap[:], EPS)
        w_sb = []
        for kt in range(KT):
            t = const.tile([128, 6 * DH], F16, tag=f"w{kt}")
            nc.sync.dma_start(t[:], wbig[bass.ts(kt, 128), :])
            w_sb.append(t)
        wout_sb = const.tile([128, D], F16, tag="wout")
        nc.sync.dma_start(wout_sb[:], wout[:, :])

        for b in range(B):
            # ---------------- Phase A: LN + scaled rows -> DRAM scratch ----
            xs_dram = dramp.tile([N, DAUG], F16, tag="xs_dram")
            for nt in range(NT):
                xt = apool.tile([128, D], F16, tag="xt")
                nc.sync.dma_start(xt[:], x_in[b, bass.ts(nt, 128), :])
                stats = spool.tile([128, 2, 6], F32, tag="stats")
                xg = xt[:].rearrange("p (s f) -> p s f", f=512)
                nc.vector.bn_stats(out=stats[:, 0, :], in_=xg[:, 0, :])
                nc.vector.bn_stats(out=stats[:, 1, :], in_=xg[:, 1, :])
                mv = spool.tile([128, 2], F32, tag="mv")
                nc.vector.bn_aggr(out=mv[:], in_=stats[:])
                lnv = spool.tile([128, 1], F32, tag="lnv")
                nc.scalar.activation(lnv[:], mv[:, 1:2],
                                     mybir.ActivationFunctionType.Ln,
                                     bias=eps_ap[:])
                rstd = spool.tile([128, 1], F32, tag="rstd")
                nc.scalar.activation(rstd[:], lnv[:],
                                     mybir.ActivationFunctionType.Exp, scale=-0.5)
                xs = apool.tile([128, DAUG], F16, tag="xs")
                nc.vector.tensor_scalar_mul(xs[:, 0:D], xt[:], rstd[:])
                # aug cols: murstd | 1 | zeros
                nc.scalar.mul(xs[:, D:D + 1], mv[:, 0:1], rstd[:])
                nc.gpsimd.memset(xs[:, D + 1:D + 2], 1.0)
                nc.gpsimd.memset(xs[:, D + 2:DAUG], 0.0)
                nc.sync.dma_start(xs_dram[bass.ts(nt, 128), :], xs[:])

            # ---------------- Phase B1: transposed loads -------------------
            xsT = []
            for kt in range(KT):
                t = xsp.tile([128, N], F16, tag=f"xsT{kt}")
                nc.sync.dma_start(t[:], xs_dram[:, bass.ts(kt, 128)],
                                  transpose=True)
                xsT.append(t)

            # ---------------- Phase B2: QKV projections --------------------
            # qTb/kTb: [128, N]; partitions 0:64 = head0, 64:128 = head1
            qTb = qkp.tile([128, N], F16, tag="qTb")
            kTb = qkp.tile([128, N], F16, tag="kTb")
            stq = qkp.tile([64, N], F16, tag="stq")     # q_h1 staging (base 0)
            stk = qkp.tile([128, N], F16, tag="stk")    # k_h0 staging (base 64)
            for h in range(HPC):
                with tc.tile_pool(name=f"qkps{b}_{h}", bufs=2,
                                  space="PSUM") as ps:
                    for c in range(4):
                        acc = ps.tile([128, 512], F32, name=f"acc{c}",
                                      tag="acc")
                        for kt in range(KT):
                            nc.tensor.matmul(
                                acc[:],
                                w_sb[kt][:, bass.ds(h * 128, 128)],
                                xsT[kt][:, bass.ts(c, 512)],
                                start=(kt == 0), stop=(kt == KT - 1))
                        if h == 0:
                            nc.vector.tensor_copy(qTb[0:64, bass.ts(c, 512)],
                                                  acc[0:64, :])
                            nc.vector.tensor_copy(stk[64:128, bass.ts(c, 512)],
                                                  acc[64:128, :])
                        else:
                            nc.vector.tensor_copy(stq[:, bass.ts(c, 512)],
                                                  acc[0:64, :])
                            nc.vector.tensor_copy(kTb[64:128, bass.ts(c, 512)],
                                                  acc[64:128, :])
            # partition-shift moves via DMA
            nc.sync.dma_start(qTb[64:128, :], stq[:, :])
            nc.sync.dma_start(kTb[0:64, :], stk[64:128, :])

            v_sb = []    # 16 tiles [128, 130]: per head 64 v cols + ones col
            vpool_cm = tc.tile_pool(name=f"vps{b}", bufs=2, space="PSUM")
            with vpool_cm as vps_pool:
              for nt in range(NT):
                va = vp.tile([128, 2 * (DH + 1)], F16, tag=f"v{nt}")
                if True:
                    acc = vps_pool.tile([128, 128], F32, name=f"vacc{nt}",
                                        tag="vacc")
                    for kt in range(KT):
                        nc.tensor.matmul(acc[:], xsT[kt][:, bass.ts(nt, 128)],
                                         w_sb[kt][:, 256:384],
                                         start=(kt == 0), stop=(kt == KT - 1))
                    for h in range(HPC):
                        nc.vector.tensor_copy(
                            va[:, bass.ds(h * (DH + 1), DH)],
                            acc[:, bass.ds(h * DH, DH)])
                nc.gpsimd.memset(va[:, DH:DH + 1], 1.0)
                nc.gpsimd.memset(va[:, 2 * DH + 1:2 * DH + 2], 1.0)
                v_sb.append(va)

            # ---------------- Phase B3: attention --------------------------
            attnT = atp.tile([128, N], F16, tag="attnT")
            for ih in range(IH):
                with tc.tile_pool(name=f"pv{b}_{ih}", bufs=2,
                                  space="PSUM") as pvps, \
                     tc.tile_pool(name=f"sps{b}_{ih}", bufs=2,
                                  space="PSUM") as sps:
                    pv = [pvps.tile([DH + 1, IW], F32, name=f"pv{h}",
                                    tag="pv") for h in range(HPC)]
                    for jt in range(NT):
                        als = []
                        for h in range(HPC):
                            al = alp.tile([128, IW], F16, tag="al")
                            nc.sync.dma_start(
                                al[:],
                                alibiT[h, bass.ts(jt, 128),
                                       bass.ds(ih * IW, IW)])
                            als.append(al)
                        sp_t = [sps.tile([128, IW], F32, name=f"sp{h}",
                                         tag="sp") for h in range(HPC)]
                        for c in range(2):
                            for h in range(HPC):
                                nc.tensor.matmul(
                                    sp_t[h][:, bass.ts(c, 512)],
                                    kTb[bass.ds(h * 64, 64), bass.ts(jt, 128)],
                                    qTb[bass.ds(h * 64, 64),
                                        bass.ds(ih * IW + c * 512, 512)],
                                    start=True, stop=False,
                                    tile_position=(h * 64, 0))
                        for h in range(HPC):
                            for c in range(2):
                                nc.tensor.matmul(
                                    sp_t[h][:, bass.ts(c, 512)],
                                    ident[:],
                                    als[h][:, bass.ts(c, 512)],
                                    start=False, stop=True)
                            p = pp.tile([128, IW], F16, tag="p")
                            nc.scalar.activation(
                                p[:], sp_t[h][:],
                                mybir.ActivationFunctionType.Exp)
                            for c in range(2):
                                nc.tensor.matmul(
                                    pv[h][:, bass.ts(c, 512)],
                                    v_sb[jt][:, bass.ds(h * (DH + 1), DH + 1)],
                                    p[:, bass.ts(c, 512)],
                                    start=(jt == 0), stop=(jt == NT - 1))
                    # fast PSUM drain: copy out, normalize later from SBUF
                    for h in range(HPC):
                        un = ep.tile([64, IW], F32, tag="un")
                        nc.vector.tensor_copy(un[:], pv[h][0:DH, :])
                        srow = ep.tile([1, IW], F32, tag="srow")
                        nc.vector.tensor_copy(srow[:], pv[h][DH:DH + 1, :])
                        sdram = dramp.tile([1, IW], F32, tag="sdram")
                        nc.sync.dma_start(sdram[:], srow[:])
                        rr = ep.tile([128, IW // 128], F32, tag="rr")
                        nc.sync.dma_start(
                            rr[:], sdram[0, :].rearrange("(p f) -> p f", p=128))
                        rr16 = ep.tile([128, IW // 128], F16, tag="rr16")
                        with nc.allow_low_precision(reason="softmax recip f16"):
                            nc.vector.reciprocal(rr16[:], rr[:])
                        rdram = dramp.tile([1, IW], F16, tag="rdram")
                        nc.sync.dma_start(
                            rdram[0, :].rearrange("(p f) -> p f", p=128),
                            rr16[:])
                        rec = ep.tile([1, IW], F16, tag="rec")
                        nc.sync.dma_start(rec[:], rdram[:])
                        rb = ep.tile([64, IW], F16, tag="rb")
                        nc.gpsimd.partition_broadcast(rb[:], rec[:],
                                                      channels=64)
                        nc.vector.tensor_mul(
                            attnT[bass.ds(h * DH, DH), bass.ds(ih * IW, IW)],
                            un[:], rb[:])

            # ---------------- Phase B4: out projection ---------------------
            with tc.tile_pool(name=f"op{b}", bufs=2,
                              space="PSUM") as ops:
              for nt in range(NT):
                if True:
                    acc = ops.tile([128, D], F32, name=f"oacc{nt}",
                                   tag="oacc")
                    for mc in range(2):
                        nc.tensor.matmul(acc[:, bass.ts(mc, 512)],
                                         attnT[:, bass.ts(nt, 128)],
                                         wout_sb[:, bass.ts(mc, 512)],
                                         start=True, stop=True)
                    ot = outsb.tile([128, D], F16, tag="ot")
                    if nt % 2 == 0:
                        nc.vector.tensor_copy(ot[:], acc[:])
                    else:
                        nc.scalar.copy(ot[:], acc[:])
                nc.sync.dma_start(outp[b, bass.ts(nt, 128), :], ot[:])

    nc.compile()
    return nc


def _get_nc():
    if "nc" not in _CACHE:
        _CACHE["nc"] = build()
    return _CACHE["nc"]


def kernel(x, alibi, w_qkv, w_out, b_out, ln_g, ln_b):
    x = np.asarray(x, dtype=np.float32)
    alibi = np.asarray(alibi, dtype=np.float32)
    w_qkv = np.asarray(w_qkv, dtype=np.float32)
    w_out = np.asarray(w_out, dtype=np.float32)
    b_out = np.asarray(b_out, dtype=np.float32)
    ln_g = np.asarray(ln_g, dtype=np.float32)
    ln_b = np.asarray(ln_b, dtype=np.float32)

    # fold LN gain + attention scale into the QKV weight; LN bias becomes an
    # extra row via the augmented contraction.
    W = w_qkv * ln_g[:, None]
    W[:, :D] *= SCALE
    c_row = ln_b @ w_qkv
    c_row[:D] *= SCALE
    colsum = W.sum(axis=0)

    x16 = x.astype(np.float16)
    in_maps = []
    for core in range(N_CORES):
        hs = [HPC * core + i for i in range(HPC)]
        cols = []
        for h in hs:
            cols.extend(range(h * DH, (h + 1) * DH))           # q
            cols.extend(range(D + h * DH, D + (h + 1) * DH))   # k
        vcols = []
        for h in hs:
            vcols.extend(range(2 * D + h * DH, 2 * D + (h + 1) * DH))
        cols = cols + vcols  # [q0|k0|q1|k1|v0|v1] -> 6*DH columns
        wbig = np.zeros((DAUG, 6 * DH), dtype=np.float32)
        wbig[:D, :] = W[:, cols]
        wbig[D, :] = -colsum[cols]
        wbig[D + 1, :] = c_row[cols]
        alT = np.ascontiguousarray(
            alibi[hs].transpose(0, 2, 1)) - np.float32(EXP_SHIFT)
        in_maps.append({
            "x": x16,
            "alibiT": alT.astype(np.float16),
            "wbig": wbig.astype(np.float16),
            "wout": w_out[hs[0] * DH: hs[0] * DH + HPC * DH, :]
                    .astype(np.float16),
        })

    nc = _get_nc()
    res = run_bass_kernel_spmd(nc, in_maps, list(range(N_CORES)),
                               trace=PROFILE)
    LAST_RESULT["exec_time_ns"] = res.exec_time_ns
    LAST_RESULT["mean_exec_time_ns"] = res.mean_exec_time_ns
    LAST_RESULT["instructions_and_trace"] = res.instructions_and_trace

    out = np.zeros((B, N, D), dtype=np.float32)
    for core in range(N_CORES):
        out += res.results[core]["outp"].astype(np.float32)
    out += b_out
    return out
